# revision 18
# baseline (speedup 1.0000x reference)
"""Particles2Grid (SPH cubic-spline splat) Trainium2 Bass kernel.

Sharding: 8 NeuronCores = (batch b in {0,1}) x (x-quarter q in {0..3}).
Each core owns output slab [32, 128, 128, 4] (x-range [32q, 32q+32)).
Host routes particles (with +-2 cell x-halo) to cores, duplicates rows
across 8-slice "phases", sorts by bz, and packs (phase, bz)-bins into
128-row tiles (shared structure across cores so one SPMD program works).

Device pipeline per core (phase-major, 8-tile chunks):
  px/py/pz reconstructed in f32 from uint8 cell bases + uint8 in-cell
  offsets (quantization err <= H/510 ~ 2e-4, well inside tolerance)
  dxa[k,xs]  = cxs - px              dy/dz analog via (b+o+0.5)*H - p
  d2[k,(oy,xs,oz)] = dxa^2 (+) dy^2 (+) dz^2      (broadcast-AP adds)
  q = ACT Sqrt(25*d2); q2 = ACT Square(q)
  W = custom-DVE relu(min(0.5 - 3*q2*(1-q), (1-q)^3))        [bf16]
  vals[k,(oy,xs,oz,c)] = W * cdat    (cdat = 2*sigma/(im*rho) * data)
  onehot[k,132] = (iota == by+2)                              [bf16]
  per (phase, z-bin unit): psum[y',(xs,oz,c)] += 5 shifted one-hot matmuls
  slab[y, xs, 4z-8:4z+12] += psum    (z-clipped, f32 accumulate)
  slab -> uint8 (q = RNE(v*127/absmax + 127), per-(y,x) block absmax);
  host decodes (q-127)*absmax/127. Integer offset keeps 0 on the
  reconstruction grid (most cells are near zero; a half-quantum offset
  would 5x the error).

Host/runtime (the axon PJRT tunnel is ONE serial in-order ~50-65MB/s
pipe — no overlap between uploads, exec, and downloads is possible, so
minimize bytes and RPC count): per call there is exactly ONE upload
(u8-packed particle data + bf16 cdat bytes in one [1024,T,14] buffer,
~5.8MB), one jitted shard_map dispatch, and ONE download ([256,128,516]
u8 with the f32 block scales embedded in the last 4 columns, ~16.9MB).
The jitted callable is built once per plan signature; donated output
buffers are recycled from the previous call (outputs fully overwritten,
first call seeds with on-device zeros); call-invariant inputs
(cxs/iota/oyc/ozc/bzf) are device-resident. Decode is threaded and
writes straight into the final [2,128,128,128,4] array.
"""

import sys

if "/opt/trn_rl_repo" not in sys.path:
    sys.path.insert(0, "/opt/trn_rl_repo")

import numpy as np

import concourse.bass as bass
import concourse.bacc as bacc
import concourse.tile as tile
from concourse import mybir

# ---------------------------------------------------------------- constants
GS = 128
H = np.float32(0.1)
SIGMA = np.float32(8.0 / (np.pi * 0.2**3))
C = 4
NCORES = 8
NPH = 4          # phases per core
NW = 16          # x-windows per batch (4 quarters x 4 phases)
B = 2
N = 100000
PK = 13          # packed upload row bytes: 5 meta + 8 cdat bf16 bytes

f32 = mybir.dt.float32
bf16 = mybir.dt.bfloat16
u8 = mybir.dt.uint8
BF16NP = mybir.dt.np(bf16)

# ------------------------------------------------------- custom DVE spline
# W = relu(min(0.5 - 3*q2*u, u^3)), u = 1-q.  (x2 folded into cdat host-side)
_SPLINE = None


def _register_spline():
    global _SPLINE
    if _SPLINE is not None:
        return _SPLINE
    from concourse.dve_spec import Spec, Src0, Src1, C0, C2, One, relu, sq, minn, lower
    from concourse.dve_ops import DveOp, OPS, CUSTOM_DVE_SPECS, _SUB_OPCODE_FOR_NAME
    from concourse.dve_uop import DveOpSpec

    name = "SPH_SPLINE_ANT"
    if name in _SUB_OPCODE_FOR_NAME:
        for op in OPS:
            if op.name == name:
                _SPLINE = op
                return op

    def spline_ref(in0, in1, s0, s1, imm2):
        q = in0.astype(np.float32)
        q2 = in1.astype(np.float32)
        u = (1.0 - q).astype(np.float32)
        return np.maximum(
            np.minimum(np.float32(imm2) - q2 * u * s0, u * u * u), 0.0
        ).astype(np.float32)

    u = One - Src0
    body = relu(minn(C2 - (Src1 * u) * C0, sq(u) * u))
    spec = Spec(body=body, reference=spline_ref)
    opcode = 1 + len(OPS)
    _SUB_OPCODE_FOR_NAME[name] = opcode
    shas = {}
    for ver in ("v3", "v4"):
        shas[ver] = DveOpSpec(
            name=name, opcode=opcode, uops=lower(spec, ver=ver), rd1_en=True
        ).sha(ver)
    op = DveOp(name, spec, subdim=False, uops_sha=shas)
    OPS.append(op)
    CUSTOM_DVE_SPECS[name] = spec
    _SPLINE = op
    return op


# ---------------------------------------------------------------- host prep
def _build_plan(locs, data, density):
    """Vectorized tile/bin plan (no heavy per-core arrays; see _fill_all)."""
    pos = np.asarray(locs[..., :3], np.float32)
    inv_mass = np.asarray(locs[..., 3], np.float32)
    data = np.asarray(data, np.float32)
    dens = np.asarray(density, np.float32)

    base = np.floor(pos / H).astype(np.int32)          # [B, N, 3]
    cdat_all = data * (np.float32(2.0) * SIGMA / (inv_mass * dens))[..., None]

    # window membership: particle in window w iff 8w-2 <= bx <= 8w+9
    # (12-wide, stride 8 -> each particle in 1 or 2 windows)
    sorted_src = []
    cuts_all = []
    counts = np.empty((B, NW, GS), np.int64)
    idx_all = np.arange(N, dtype=np.int32)
    for b in range(B):
        bx = base[b, :, 0]
        bz = base[b, :, 2]
        w_lo = np.clip((bx - 2) // 8, 0, NW - 1)
        w_hi = np.clip((bx + 2) // 8, 0, NW - 1)
        dup = np.nonzero(w_hi > w_lo)[0].astype(np.int32)
        pi = np.concatenate([idx_all, dup])
        pw = np.concatenate([w_lo, w_hi[dup]])
        key = (pw * GS + bz[pi]).astype(np.int32)
        order = np.argsort(key, kind="stable")
        sorted_src.append(pi[order])
        ct = np.searchsorted(key[order], np.arange(NW * GS + 1))
        cuts_all.append(ct)
        counts[b] = np.diff(ct).reshape(NW, GS)

    # caps[p, z] = max over (b, q) of per-(core,phase,z) count, padded to 32
    caps = counts.reshape(B, 4, NPH, GS).max(axis=(0, 1))
    caps = ((caps + 31) // 32) * 32

    # pack units into tiles per phase: units >32 open fresh tiles (base 0),
    # 32-units first-fit into gaps at bases {32,64,96}
    phase_units = []   # per phase: list of (z, tile_global, r0, nrows, zoff)
    phase_tiles = []   # per phase: (t_lo, t_hi)
    fills = []         # per phase: vectorized fill helper arrays
    Tg = 0
    for p in range(NPH):
        units = []
        for z in range(GS):
            cp = int(caps[p, z])
            off = 0
            while cp > 0:
                take = min(128, cp)
                units.append((z, take, off))
                off += take
                cp -= take
        units.sort(key=lambda u: -u[1])
        tiles_fill = []
        placed = []
        for z, size, zoff in units:
            if size > 32:
                placed.append((z, len(tiles_fill), 0, size, zoff))
                tiles_fill.append(size)
            else:
                for t in range(len(tiles_fill)):
                    if tiles_fill[t] + 32 <= 128:
                        placed.append((z, t, tiles_fill[t], 32, zoff))
                        tiles_fill[t] += 32
                        break
                else:
                    placed.append((z, len(tiles_fill), 0, 32, zoff))
                    tiles_fill.append(32)
        nt = len(tiles_fill)
        pu = [(z, Tg + t, r0, nr, zoff) for (z, t, r0, nr, zoff) in placed]
        phase_units.append(pu)
        phase_tiles.append((Tg, Tg + nt))
        Tg += nt

        uz = np.array([u[0] for u in pu], np.int64)
        ut = np.array([u[1] for u in pu], np.int64)
        ur0 = np.array([u[2] for u in pu], np.int64)
        unr = np.array([u[3] for u in pu], np.int64)
        uzf = np.array([u[4] for u in pu], np.int64)
        tot = int(unr.sum())
        rep = np.repeat(np.arange(len(pu)), unr)
        starts = np.concatenate([[0], np.cumsum(unr)[:-1]])
        within = np.arange(tot) - starts[rep]
        fills.append(
            dict(
                gslot=ut[rep] * 128 + ur0[rep] + within,
                binl=uz[rep],
                srcoff=uzf[rep] + within,
            )
        )

    sig = (
        Tg,
        tuple(phase_tiles),
        tuple(tuple(u[:4] for u in ph) for ph in phase_units),
    )
    return dict(
        T=Tg,
        phase_tiles=phase_tiles,
        phase_units=phase_units,
        fills=fills,
        sorted_src=sorted_src,
        cuts_all=cuts_all,
        base=base,
        pos=pos,
        cdat_all=cdat_all,
        sig=sig,
    )


def _fill_all(plan):
    """One packed upload buffer [8*128, T, PK] u8:
    cols 0..2 = quantized in-cell offsets, 3 = bx (255=pad), 4 = by+2,
    5..12 = cdat bf16 bytes. Single fused scatter across all (core,phase)
    pairs."""
    T = plan["T"]
    base = plan["base"]
    pos = plan["pos"]
    cdat_all = plan["cdat_all"]

    # gather (dst_slot, src_particle) pairs across all 32 (core, phase) combos
    dsts, srcs = [], []
    stride = T * 128
    for c in range(NCORES):
        b, q = c // 4, c % 4
        ct = plan["cuts_all"][b]
        for p in range(NPH):
            f = plan["fills"][p]
            wbin = (4 * q + p) * GS + f["binl"]
            srcp = ct[wbin] + f["srcoff"]
            valid = srcp < ct[wbin + 1]
            dsts.append(f["gslot"][valid] + c * stride)
            srcs.append(plan["sorted_src"][b][srcp[valid]] + b * N)
    dst = np.concatenate(dsts)
    src = np.concatenate(srcs)

    pk = np.zeros((NCORES * T * 128, PK), np.uint8)
    pk[:, 3] = 255
    qscale = np.float32(255.0) / H
    posf = pos.reshape(B * N, 3)
    basef = base.reshape(B * N, 3)
    bs = basef[src]
    off = posf[src] - bs.astype(np.float32) * H
    pk[dst, :3] = np.clip(np.rint(off * qscale), 0, 255).astype(np.uint8)
    pk[dst, 3] = bs[:, 0]
    pk[dst, 4] = bs[:, 1] + 2
    pk[:, 5:13].view(BF16NP)[dst] = (
        cdat_all.reshape(B * N, C)[src].astype(BF16NP)
    )

    return np.ascontiguousarray(
        pk.reshape(NCORES, T, 128, PK).transpose(0, 2, 1, 3)
    ).reshape(NCORES * 128, T, PK)


def _consts(plan):
    """Call-invariant (per-signature) inputs, device-resident."""
    T = plan["T"]
    bz_flat = np.zeros(T * 128, np.float32)
    for f in plan["fills"]:
        bz_flat[f["gslot"]] = f["binl"]
    bzf = np.ascontiguousarray(
        np.broadcast_to(bz_flat.reshape(T, 128).T[None], (NCORES, 128, T))
    ).reshape(NCORES * 128, T)

    cxs = np.empty((NCORES, 128, 32), np.float32)
    for c in range(NCORES):
        X0 = 32 * (c % 4)
        cxs[c] = ((np.arange(32, dtype=np.float32) + X0) + np.float32(0.5)) * H
    cxs = cxs.reshape(NCORES * 128, 32)

    iota = np.ascontiguousarray(
        np.broadcast_to(
            np.arange(132, dtype=np.float32), (NCORES * 128, 132)
        )
    ).astype(BF16NP)
    oc = np.ascontiguousarray(
        np.broadcast_to(
            np.arange(-2, 3).astype(np.float32) + 0.5, (NCORES * 128, 5)
        )
    )
    return dict(bzf=bzf, cxs=cxs, iota=iota, oyc=oc, ozc=oc.copy())


# ------------------------------------------------------------ bass program
CH = 8  # tiles per chunk


def _build_nc(plan):
    spline = _register_spline()
    T = plan["T"]
    nc = bacc.Bacc("TRN2", target_bir_lowering=False, debug=False, num_devices=NCORES)

    di = {}
    di["pk"] = nc.dram_tensor("pk", [128, T, PK], u8, kind="ExternalInput")
    di["bzf"] = nc.dram_tensor("bzf", [128, T], f32, kind="ExternalInput")
    di["cxs"] = nc.dram_tensor("cxs", [128, 32], f32, kind="ExternalInput")
    di["iota"] = nc.dram_tensor("iota", [128, 132], bf16, kind="ExternalInput")
    di["oyc"] = nc.dram_tensor("oyc", [128, 5], f32, kind="ExternalInput")
    di["ozc"] = nc.dram_tensor("ozc", [128, 5], f32, kind="ExternalInput")
    OUTQ = nc.dram_tensor("OUTQ", [32, 128, 516], u8, kind="ExternalOutput")

    Sq = mybir.ActivationFunctionType.Square
    Sqrt = mybir.ActivationFunctionType.Sqrt
    Copy = mybir.ActivationFunctionType.Copy
    AOp = mybir.AluOpType

    with tile.TileContext(nc) as tc:
        with (
            tc.tile_pool(name="ins", bufs=1) as ins,
            tc.tile_pool(name="work", bufs=2) as wk,
            tc.tile_pool(name="slabp", bufs=2) as slabp,
            tc.tile_pool(name="psum", bufs=8, space="PSUM") as psp,
        ):
            # resident inputs
            sb = {}
            sb["pk"] = ins.tile([128, T, PK], u8, tag="pk", name="pk")
            nc.sync.dma_start(sb["pk"][:], di["pk"][:])
            for nm, w, dt_ in (
                ("bzf", T, f32), ("cxs", 32, f32), ("iota", 132, bf16),
                ("oyc", 5, f32), ("ozc", 5, f32),
            ):
                sb[nm] = ins.tile([128, w], dt_, tag=nm, name=nm + "_sb")
                nc.sync.dma_start(sb[nm][:], di[nm][:])

            # unpack cdat bf16 bytes into a contiguous tile (SBUF->SBUF DMA)
            sb["cdat"] = ins.tile([128, T, C], bf16, tag="cdat", name="cdat_sb")
            nc.sync.dma_start(
                sb["cdat"][:].bitcast(u8), sb["pk"][:, :, 5:13]
            )
            # decode u8 pack -> f32 positions
            pqs = ins.tile([128, T, 3], f32, tag="pqs", name="pqs_sb")
            nc.scalar.activation(
                pqs[:],
                sb["pk"][:, :, 0:3],
                Copy,
                scale=float(H / np.float32(255.0)),
            )
            sb["by2f"] = ins.tile([128, T], f32, tag="by2f", name="by2f_sb")
            nc.scalar.copy(sb["by2f"][:], sb["pk"][:, :, 4])
            sb["by2b"] = ins.tile([128, T], bf16, tag="by2b", name="by2b_sb")
            nc.scalar.copy(sb["by2b"][:], sb["pk"][:, :, 4])
            sb["byf"] = ins.tile([128, T], f32, tag="byf", name="byf_sb")
            nc.vector.tensor_scalar(
                out=sb["byf"][:], in0=sb["by2f"][:],
                scalar1=-2.0, scalar2=None, op0=AOp.add,
            )
            # px = bx*H + qx*(H/255); py = (by2-2)*H + qy*(H/255); pz analog
            bxH = ins.tile([128, T], f32, tag="bxH", name="bxH_sb")
            nc.scalar.activation(
                bxH[:], sb["pk"][:, :, 3], Copy, scale=float(H)
            )
            byH = ins.tile([128, T], f32, tag="byH", name="byH_sb")
            nc.vector.tensor_scalar(
                out=byH[:], in0=sb["by2f"][:],
                scalar1=float(H), scalar2=float(-2.0 * H),
                op0=AOp.mult, op1=AOp.add,
            )
            bzH = ins.tile([128, T], f32, tag="bzH", name="bzH_sb")
            nc.vector.tensor_scalar(
                out=bzH[:], in0=sb["bzf"][:],
                scalar1=float(H), scalar2=None, op0=AOp.mult,
            )
            for nm, bt, cidx in (("px", bxH, 0), ("py", byH, 1), ("pz", bzH, 2)):
                sb[nm] = ins.tile([128, T], f32, tag=nm, name=nm)
                nc.vector.tensor_tensor(
                    out=sb[nm][:], in0=bt[:], in1=pqs[:, :, cidx], op=AOp.add
                )

            for p in range(NPH):
                t_lo, t_hi = plan["phase_tiles"][p]
                ntile = t_hi - t_lo
                slab = slabp.tile([128, 8, 512], f32, tag="slab")
                nc.gpsimd.memset(slab[:], 0.0)

                # group units by chunk
                units_by_chunk = {}
                for z, t, r0, nr, zoff in plan["phase_units"][p]:
                    ci = (t - t_lo) // CH
                    units_by_chunk.setdefault(ci, []).append((z, t, r0, nr))

                nchunk = (ntile + CH - 1) // CH
                for ci in range(nchunk):
                    c_lo = t_lo + ci * CH
                    cw = min(CH, t_hi - c_lo)
                    sl = slice(c_lo, c_lo + cw)

                    # ---- A: axis deltas
                    dxa = wk.tile([128, CH, 8], f32, tag="dxa")
                    nc.vector.tensor_tensor(
                        out=dxa[:, :cw],
                        in0=sb["cxs"][:, None, 8 * p:8 * p + 8].to_broadcast(
                            [128, cw, 8]
                        ),
                        in1=sb["px"][:, sl, None].to_broadcast([128, cw, 8]),
                        op=AOp.subtract,
                    )
                    dxa2 = wk.tile([128, CH, 8], f32, tag="dxa2")
                    nc.scalar.activation(dxa2[:, :cw], dxa[:, :cw], Sq)

                    ty = wk.tile([128, CH, 5], f32, tag="ty")
                    nc.vector.tensor_tensor(
                        out=ty[:, :cw],
                        in0=sb["byf"][:, sl, None].to_broadcast([128, cw, 5]),
                        in1=sb["oyc"][:, None, :].to_broadcast([128, cw, 5]),
                        op=AOp.add,
                    )
                    dy = wk.tile([128, CH, 5], f32, tag="dy")
                    nc.vector.scalar_tensor_tensor(
                        out=dy[:, :cw],
                        in0=ty[:, :cw],
                        scalar=float(H),
                        in1=sb["py"][:, sl, None].to_broadcast([128, cw, 5]),
                        op0=AOp.mult,
                        op1=AOp.subtract,
                    )
                    dy2 = wk.tile([128, CH, 5], f32, tag="dy2")
                    nc.scalar.activation(dy2[:, :cw], dy[:, :cw], Sq)

                    tz = wk.tile([128, CH, 5], f32, tag="tz")
                    nc.vector.tensor_tensor(
                        out=tz[:, :cw],
                        in0=sb["bzf"][:, sl, None].to_broadcast([128, cw, 5]),
                        in1=sb["ozc"][:, None, :].to_broadcast([128, cw, 5]),
                        op=AOp.add,
                    )
                    dz = wk.tile([128, CH, 5], f32, tag="dz")
                    nc.vector.scalar_tensor_tensor(
                        out=dz[:, :cw],
                        in0=tz[:, :cw],
                        scalar=float(H),
                        in1=sb["pz"][:, sl, None].to_broadcast([128, cw, 5]),
                        op0=AOp.mult,
                        op1=AOp.subtract,
                    )
                    dz2 = wk.tile([128, CH, 5], f32, tag="dz2")
                    nc.scalar.activation(dz2[:, :cw], dz[:, :cw], Sq)

                    # ---- B: d2 in (oy, xs, oz) order
                    tyx = wk.tile([128, CH, 5, 8], f32, tag="tyx")
                    nc.vector.tensor_tensor(
                        out=tyx[:, :cw],
                        in0=dy2[:, :cw, :, None].to_broadcast([128, cw, 5, 8]),
                        in1=dxa2[:, :cw, None, :].to_broadcast([128, cw, 5, 8]),
                        op=AOp.add,
                    )
                    d2 = wk.tile([128, CH, 40, 5], f32, tag="d2")
                    nc.vector.tensor_tensor(
                        out=d2[:, :cw],
                        in0=tyx[:, :cw].rearrange("p t a b -> p t (a b)")[
                            :, :, :, None
                        ].to_broadcast([128, cw, 40, 5]),
                        in1=dz2[:, :cw, None, :].to_broadcast([128, cw, 40, 5]),
                        op=AOp.add,
                    )

                    # ---- C: q on ACT
                    d2f = d2[:, :cw].rearrange("p t a b -> p (t a b)")
                    qt = wk.tile([128, CH, 200], f32, tag="qt")
                    qf = qt[:, :cw].rearrange("p t s -> p (t s)")
                    nc.scalar.activation(qf, d2f, Sqrt, scale=25.0)
                    # ---- D: spline -> W bf16 (q^2 == 25*d2 exactly)
                    Wt = wk.tile([128, CH, 200], bf16, tag="Wt")
                    nc.vector._custom_dve(
                        spline,
                        out=Wt[:, :cw].rearrange("p t s -> p (t s)"),
                        in0=qf,
                        in1=d2f,
                        s0=75.0,
                        s1=0.0,
                        imm2=0.5,
                    )

                    # ---- E: vals[k,t,c,spl] = W * cdat_c  (single broadcast
                    # tensor_tensor per chunk; all-bf16)
                    vals = wk.tile([128, CH, C, 200], bf16, tag="vals")
                    nc.vector.tensor_tensor(
                        out=vals[:, :cw],
                        in0=Wt[:, :cw, None, :].to_broadcast(
                            [128, cw, C, 200]
                        ),
                        in1=sb["cdat"][:, sl, :, None].to_broadcast(
                            [128, cw, C, 200]
                        ),
                        op=AOp.mult,
                    )

                    # ---- onehot (single broadcast is_equal per chunk)
                    oh = wk.tile([128, CH, 132], bf16, tag="oh")
                    nc.vector.tensor_tensor(
                        out=oh[:, :cw],
                        in0=sb["iota"][:, None, :].to_broadcast(
                            [128, cw, 132]
                        ),
                        in1=sb["by2b"][:, sl, None].to_broadcast(
                            [128, cw, 132]
                        ),
                        op=AOp.is_equal,
                    )

                    # ---- F: matmuls + evac per unit
                    for z, t, r0, nr in units_by_chunk.get(ci, []):
                        tl = t - c_lo
                        ps = psp.tile([128, 160], f32, tag="ps", name="ps")
                        for oyi in range(5):
                            c0 = 2 - (oyi - 2)
                            nc.tensor.matmul(
                                out=ps[:],
                                lhsT=oh[r0:r0 + nr, tl, c0:c0 + 128],
                                rhs=vals[
                                    r0:r0 + nr, tl, :,
                                    40 * oyi:40 * (oyi + 1)
                                ],
                                start=(oyi == 0),
                                stop=(oyi == 4),
                                tile_position=(r0, 0) if r0 >= 96 else None,
                            )
                        # evac with z-clip (cell granularity)
                        oz_lo = max(0, (8 - 4 * z) // 4)
                        oz_hi = min(5, (512 - (4 * z - 8)) // 4)
                        nz = oz_hi - oz_lo
                        zlo = 4 * z - 8 + 4 * oz_lo
                        sview = slab[:, :, zlo:zlo + 4 * nz].rearrange(
                            "p x (w c) -> p x w c", c=4
                        )
                        psr = ps[:].rearrange("p (c x w) -> p c x w", c=4, x=8)
                        pview = psr[:, :, :, oz_lo:oz_hi].rearrange(
                            "p c x w -> p x w c"
                        )
                        nc.vector.tensor_tensor(
                            out=sview, in0=sview, in1=pview, op=AOp.add
                        )

                # ---- phase out: int8 quantize with per-(y,x) block scale.
                # q = RNE(v * 127/absmax + 127)  (convert is RNE, probed);
                # host decodes v = (q - 127) * absmax/127. Integer offset so
                # the reconstruction grid contains 0 exactly (most grid cells
                # are near zero; a half-quantum offset would 5x the error).
                absm = slabp.tile([128, 8], f32, tag="absm")
                nc.vector.tensor_reduce(
                    out=absm[:], in_=slab[:], axis=mybir.AxisListType.X,
                    op=AOp.max, apply_absolute_value=True,
                )
                absc = slabp.tile([128, 8], f32, tag="absc")
                nc.vector.tensor_scalar(
                    out=absc[:], in0=absm[:],
                    scalar1=1e-30, scalar2=None, op0=AOp.max,
                )
                scl = slabp.tile([128, 8], f32, tag="scl")
                nc.vector.reciprocal(scl[:], absc[:])
                nc.vector.tensor_scalar(
                    out=scl[:], in0=scl[:],
                    scalar1=127.0, scalar2=None, op0=AOp.mult,
                )
                yt = slabp.tile([128, 8, 512], f32, tag="yt")
                nc.vector.tensor_tensor(
                    out=yt[:], in0=slab[:],
                    in1=scl[:, :, None].to_broadcast([128, 8, 512]),
                    op=AOp.mult,
                )
                q8 = slabp.tile([128, 8, 512], u8, tag="q8")
                nc.scalar.activation(
                    q8[:].rearrange("p x z -> p (x z)"),
                    yt[:].rearrange("p x z -> p (x z)"),
                    Copy, bias=127.0,
                )
                for xs in range(8):
                    nc.sync.dma_start(
                        out=OUTQ[8 * p + xs, :, 0:512], in_=q8[:, xs, :]
                    )
                # scales ride in the last 4 byte-columns of OUTQ
                nc.sync.dma_start(
                    out=OUTQ[8 * p:8 * p + 8, :, 512:516].rearrange(
                        "x p c -> p x c"
                    ),
                    in_=absm[:].bitcast(u8).rearrange("p (x c) -> p x c", c=4),
                )
    nc.compile()
    return nc


# ------------------------------------------------------------------ runner
def _make_runner(nc, consts):
    """Cached PJRT runner: replicates bass2jax.run_bass_via_pjrt but hoists
    the jitted shard_map (trace/compile once), recycles the previous call's
    output buffers as the donated outputs (outputs are fully overwritten),
    and keeps call-invariant inputs device-resident. dispatch() is async:
    it returns a dict of global jax Arrays (fetch with np.asarray)."""
    import jax
    import jax.numpy as jnp
    from jax.experimental.shard_map import shard_map
    from jax.sharding import Mesh, PartitionSpec, NamedSharding
    from concourse import bass2jax
    from concourse.bass2jax import _bass_exec_p, partition_id_tensor

    bass2jax.install_neuronx_cc_hook()
    assert nc.dbg_addr is None, "build with debug=False"
    pname = nc.partition_id_tensor.name if nc.partition_id_tensor is not None else None

    in_names, out_names, out_avals = [], [], []
    for alloc in nc.m.functions[0].allocations:
        if not isinstance(alloc, mybir.MemoryLocationSet):
            continue
        name = alloc.memorylocations[0].name
        if alloc.kind == "ExternalInput":
            if name != pname:
                in_names.append(name)
        elif alloc.kind == "ExternalOutput":
            out_names.append(name)
            out_avals.append(
                jax.core.ShapedArray(
                    tuple(alloc.tensor_shape), mybir.dt.np(alloc.dtype)
                )
            )
    n_params = len(in_names)
    n_outs = len(out_names)
    bind_names = tuple(in_names + out_names + ([pname] if pname else []))
    donate = tuple(range(n_params, n_params + n_outs))

    def _body(*args):
        operands = list(args)
        if pname is not None:
            operands.append(partition_id_tensor())
        outs = _bass_exec_p.bind(
            *operands,
            out_avals=tuple(out_avals),
            in_names=bind_names,
            out_names=tuple(out_names),
            lowering_input_output_aliases=(),
            sim_require_finite=True,
            sim_require_nnan=True,
            nc=nc,
        )
        return tuple(outs)

    devices = jax.devices()[:NCORES]
    assert len(devices) == NCORES
    mesh = Mesh(np.asarray(devices), ("core",))
    P_ = PartitionSpec("core")
    sharded = jax.jit(
        shard_map(
            _body,
            mesh=mesh,
            in_specs=(P_,) * (n_params + n_outs),
            out_specs=(P_,) * n_outs,
            check_rep=False,
        ),
        donate_argnums=donate,
        keep_unused=True,
    )
    gsh = NamedSharding(mesh, P_)
    zshapes = [(NCORES * a.shape[0], *a.shape[1:]) for a in out_avals]
    zdtypes = [a.dtype for a in out_avals]
    zfn = jax.jit(
        lambda: tuple(jnp.zeros(s, d) for s, d in zip(zshapes, zdtypes)),
        out_shardings=(gsh,) * n_outs,
    )

    const_dev = {nm: jax.device_put(arr, gsh) for nm, arr in consts.items()}
    state = {"prev": None}

    def dispatch(var_inputs):
        dev_in = {
            nm: jax.device_put(a, gsh) for nm, a in var_inputs.items()
        }
        args = [
            const_dev[nm] if nm in const_dev else dev_in[nm]
            for nm in in_names
        ]
        seeds = state["prev"] if state["prev"] is not None else zfn()
        outs = sharded(*args, *seeds)
        state["prev"] = outs
        return {nm: o for nm, o in zip(out_names, outs)}

    return dispatch


_DEC_LUT = (np.arange(256, dtype=np.float32) - np.float32(127.0))


def _decode(arr, out6, ex):
    """Threaded decode of [256,128,516] u8 (q8 + embedded f32 scales) into
    out6 = out viewed as [2,4,32,128,512]. Returns futures."""
    scales = arr[:, :, 512:516].copy().view(np.float32)[..., 0]  # [256,128]
    fac = (scales.astype(np.float64) / 127.0).astype(np.float32)[..., None]

    def conv(ci):
        b, qq = ci // 4, ci % 4
        r = slice(ci * 32, ci * 32 + 32)
        blk = _DEC_LUT[arr[r, :, :512]]          # u8 gather -> f32
        blk *= fac[r]
        out6[b, qq] = blk

    return [ex.submit(conv, ci) for ci in range(NCORES)]


# ------------------------------------------------------------------ driver
_CACHE = {}


def kernel(locs, data, density):
    import time as _time
    from concurrent.futures import ThreadPoolExecutor

    t0 = _time.time()
    locs = np.asarray(locs)
    data = np.asarray(data)
    density = np.asarray(density)
    plan = _build_plan(locs, data, density)
    t1 = _time.time()
    entry = _CACHE.get(plan["sig"])
    if entry is None:
        entry = {"dispatch": _make_runner(_build_nc(plan), _consts(plan))}
        _CACHE[plan["sig"]] = entry
    t2 = _time.time()

    pk = _fill_all(plan)
    outs = entry["dispatch"]({"pk": pk})
    t3 = _time.time()

    out = np.empty((B, GS, GS, GS, C), np.float32)
    out6 = out.reshape(2, 4, 32, 128, 512)
    arr = np.asarray(outs["OUTQ"])               # ONE gather, blocks on exec
    t4 = _time.time()
    with ThreadPoolExecutor(NCORES) as ex:
        for f in _decode(arr, out6, ex):
            f.result()
    t5 = _time.time()
    print(
        f"[kernel] plan={t1-t0:.3f}s build={t2-t1:.3f}s fill+disp={t3-t2:.3f}s "
        f"gather={t4-t3:.3f}s dec={t5-t4:.3f}s T={plan['T']}"
    )
    return out


# revision 21
# speedup vs baseline: 1.1669x; 1.1669x over previous
"""Particles2Grid (SPH cubic-spline splat) Trainium2 Bass kernel.

Sharding: 8 NeuronCores = (batch b in {0,1}) x (x-quarter q in {0..3}).
Each core owns output slab [32, 128, 128, 4] (x-range [32q, 32q+32)).
Host routes particles (with +-2 cell x-halo) to cores, duplicates rows
across 8-slice "phases", sorts by bz, and packs (phase, bz)-bins into
128-row tiles (shared structure across cores so one SPMD program works).

Device pipeline per core (phase-major, 8-tile chunks):
  px/py/pz reconstructed in f32 from uint8 cell bases + uint8 in-cell
  offsets (quantization err <= H/510 ~ 2e-4, well inside tolerance)
  dxa[k,xs]  = cxs - px              dy/dz analog via (b+o+0.5)*H - p
  d2[k,(oy,xs,oz)] = dxa^2 (+) dy^2 (+) dz^2      (broadcast-AP adds)
  q = ACT Sqrt(25*d2); q2 = ACT Square(q)
  W = custom-DVE relu(min(0.5 - 3*q2*(1-q), (1-q)^3))        [bf16]
  vals[k,(oy,xs,oz,c)] = W * cdat    (cdat = 2*sigma/(im*rho) * data)
  onehot[k,132] = (iota == by+2)                              [bf16]
  per (phase, z-bin unit): psum[y',(xs,oz,c)] += 5 shifted one-hot matmuls
  slab[y, xs, 4z-8:4z+12] += psum    (z-clipped, f32 accumulate)
  slab -> uint8 (q = RNE(v*127/absmax + 127), per-(y,x) block absmax);
  host decodes (q-127)*absmax/127. Integer offset keeps 0 on the
  reconstruction grid (most cells are near zero; a half-quantum offset
  would 5x the error).

Host/runtime (the axon PJRT tunnel is ONE serial in-order ~50-65MB/s
pipe — no overlap between uploads, exec, and downloads is possible, so
minimize bytes and RPC count): per call there is exactly ONE upload
(u8-packed particle data + bf16 cdat bytes in one [1024,T,14] buffer,
~5.8MB), one jitted shard_map dispatch, and ONE download ([256,128,516]
u8 with the f32 block scales embedded in the last 4 columns, ~16.9MB).
The jitted callable is built once per plan signature; donated output
buffers are recycled from the previous call (outputs fully overwritten,
first call seeds with on-device zeros); call-invariant inputs
(cxs/iota/oyc/ozc/bzf) are device-resident. Decode is threaded and
writes straight into the final [2,128,128,128,4] array.
"""

import sys

if "/opt/trn_rl_repo" not in sys.path:
    sys.path.insert(0, "/opt/trn_rl_repo")

import numpy as np

import concourse.bass as bass
import concourse.bacc as bacc
import concourse.tile as tile
from concourse import mybir

# ---------------------------------------------------------------- constants
GS = 128
H = np.float32(0.1)
SIGMA = np.float32(8.0 / (np.pi * 0.2**3))
C = 4
NCORES = 8
NPH = 4          # phases per core
NW = 16          # x-windows per batch (4 quarters x 4 phases)
B = 2
N = 100000
PK = 13          # packed upload row bytes: 5 meta + 8 cdat bf16 bytes

f32 = mybir.dt.float32
bf16 = mybir.dt.bfloat16
u8 = mybir.dt.uint8
BF16NP = mybir.dt.np(bf16)

# ------------------------------------------------------- custom DVE spline
# W = relu(min(0.5 - 3*q2*u, u^3)), u = 1-q.  (x2 folded into cdat host-side)
_SPLINE = None


def _register_spline():
    global _SPLINE
    if _SPLINE is not None:
        return _SPLINE
    from concourse.dve_spec import Spec, Src0, Src1, C0, C2, One, relu, sq, minn, lower
    from concourse.dve_ops import DveOp, OPS, CUSTOM_DVE_SPECS, _SUB_OPCODE_FOR_NAME
    from concourse.dve_uop import DveOpSpec

    name = "SPH_SPLINE_ANT"
    if name in _SUB_OPCODE_FOR_NAME:
        for op in OPS:
            if op.name == name:
                _SPLINE = op
                return op

    def spline_ref(in0, in1, s0, s1, imm2):
        q = in0.astype(np.float32)
        q2 = in1.astype(np.float32)
        u = (1.0 - q).astype(np.float32)
        return np.maximum(
            np.minimum(np.float32(imm2) - q2 * u * s0, u * u * u), 0.0
        ).astype(np.float32)

    u = One - Src0
    body = relu(minn(C2 - (Src1 * u) * C0, sq(u) * u))
    spec = Spec(body=body, reference=spline_ref)
    opcode = 1 + len(OPS)
    _SUB_OPCODE_FOR_NAME[name] = opcode
    shas = {}
    for ver in ("v3", "v4"):
        shas[ver] = DveOpSpec(
            name=name, opcode=opcode, uops=lower(spec, ver=ver), rd1_en=True
        ).sha(ver)
    op = DveOp(name, spec, subdim=False, uops_sha=shas)
    OPS.append(op)
    CUSTOM_DVE_SPECS[name] = spec
    _SPLINE = op
    return op


# ---------------------------------------------------------------- host prep
def _build_plan(locs, data, density):
    """Vectorized tile/bin plan (no heavy per-core arrays; see _fill_all)."""
    pos = np.asarray(locs[..., :3], np.float32)
    inv_mass = np.asarray(locs[..., 3], np.float32)
    data = np.asarray(data, np.float32)
    dens = np.asarray(density, np.float32)

    base = np.floor(pos / H).astype(np.int32)          # [B, N, 3]
    cdat_all = data * (np.float32(2.0) * SIGMA / (inv_mass * dens))[..., None]

    # window membership: particle in window w iff 8w-2 <= bx <= 8w+9
    # (12-wide, stride 8 -> each particle in 1 or 2 windows)
    sorted_src = []
    cuts_all = []
    counts = np.empty((B, NW, GS), np.int64)
    idx_all = np.arange(N, dtype=np.int32)
    for b in range(B):
        bx = base[b, :, 0]
        bz = base[b, :, 2]
        w_lo = np.clip((bx - 2) // 8, 0, NW - 1)
        w_hi = np.clip((bx + 2) // 8, 0, NW - 1)
        dup = np.nonzero(w_hi > w_lo)[0].astype(np.int32)
        pi = np.concatenate([idx_all, dup])
        pw = np.concatenate([w_lo, w_hi[dup]])
        key = (pw * GS + bz[pi]).astype(np.int32)
        order = np.argsort(key, kind="stable")
        sorted_src.append(pi[order])
        ct = np.searchsorted(key[order], np.arange(NW * GS + 1))
        cuts_all.append(ct)
        counts[b] = np.diff(ct).reshape(NW, GS)

    # caps[p, z] = max over (b, q) of per-(core,phase,z) count, padded to 32
    caps = counts.reshape(B, 4, NPH, GS).max(axis=(0, 1))
    caps = ((caps + 31) // 32) * 32

    # pack units into tiles per phase: units >32 open fresh tiles (base 0),
    # 32-units first-fit into gaps at bases {32,64,96}
    phase_units = []   # per phase: list of (z, tile_global, r0, nrows, zoff)
    phase_tiles = []   # per phase: (t_lo, t_hi)
    fills = []         # per phase: vectorized fill helper arrays
    Tg = 0
    for p in range(NPH):
        units = []
        for z in range(GS):
            cp = int(caps[p, z])
            off = 0
            while cp > 0:
                take = min(128, cp)
                units.append((z, take, off))
                off += take
                cp -= take
        units.sort(key=lambda u: -u[1])
        tiles_fill = []
        placed = []
        for z, size, zoff in units:
            if size > 32:
                placed.append((z, len(tiles_fill), 0, size, zoff))
                tiles_fill.append(size)
            else:
                for t in range(len(tiles_fill)):
                    if tiles_fill[t] + 32 <= 128:
                        placed.append((z, t, tiles_fill[t], 32, zoff))
                        tiles_fill[t] += 32
                        break
                else:
                    placed.append((z, len(tiles_fill), 0, 32, zoff))
                    tiles_fill.append(32)
        nt = len(tiles_fill)
        pu = [(z, Tg + t, r0, nr, zoff) for (z, t, r0, nr, zoff) in placed]
        phase_units.append(pu)
        phase_tiles.append((Tg, Tg + nt))
        Tg += nt

        uz = np.array([u[0] for u in pu], np.int64)
        ut = np.array([u[1] for u in pu], np.int64)
        ur0 = np.array([u[2] for u in pu], np.int64)
        unr = np.array([u[3] for u in pu], np.int64)
        uzf = np.array([u[4] for u in pu], np.int64)
        tot = int(unr.sum())
        rep = np.repeat(np.arange(len(pu)), unr)
        starts = np.concatenate([[0], np.cumsum(unr)[:-1]])
        within = np.arange(tot) - starts[rep]
        fills.append(
            dict(
                gslot=ut[rep] * 128 + ur0[rep] + within,
                binl=uz[rep],
                srcoff=uzf[rep] + within,
            )
        )

    sig = (
        Tg,
        tuple(phase_tiles),
        tuple(tuple(u[:4] for u in ph) for ph in phase_units),
    )
    return dict(
        T=Tg,
        phase_tiles=phase_tiles,
        phase_units=phase_units,
        fills=fills,
        sorted_src=sorted_src,
        cuts_all=cuts_all,
        base=base,
        pos=pos,
        cdat_all=cdat_all,
        sig=sig,
    )


def _fill_all(plan):
    """One packed upload buffer [8*128, T, PK] u8:
    cols 0..2 = quantized in-cell offsets, 3 = bx (255=pad), 4 = by+2,
    5..12 = cdat bf16 bytes. Single fused scatter across all (core,phase)
    pairs."""
    T = plan["T"]
    base = plan["base"]
    pos = plan["pos"]
    cdat_all = plan["cdat_all"]

    # gather (dst_slot, src_particle) pairs across all 32 (core, phase) combos
    dsts, srcs = [], []
    stride = T * 128
    for c in range(NCORES):
        b, q = c // 4, c % 4
        ct = plan["cuts_all"][b]
        for p in range(NPH):
            f = plan["fills"][p]
            wbin = (4 * q + p) * GS + f["binl"]
            srcp = ct[wbin] + f["srcoff"]
            valid = srcp < ct[wbin + 1]
            dsts.append(f["gslot"][valid] + c * stride)
            srcs.append(plan["sorted_src"][b][srcp[valid]] + b * N)
    dst = np.concatenate(dsts)
    src = np.concatenate(srcs)

    pk = np.zeros((NCORES * T * 128, PK), np.uint8)
    pk[:, 3] = 255
    qscale = np.float32(255.0) / H
    posf = pos.reshape(B * N, 3)
    basef = base.reshape(B * N, 3)
    bs = basef[src]
    off = posf[src] - bs.astype(np.float32) * H
    pk[dst, :3] = np.clip(np.rint(off * qscale), 0, 255).astype(np.uint8)
    pk[dst, 3] = bs[:, 0]
    pk[dst, 4] = bs[:, 1] + 2
    pk[:, 5:13].view(BF16NP)[dst] = (
        cdat_all.reshape(B * N, C)[src].astype(BF16NP)
    )

    return np.ascontiguousarray(
        pk.reshape(NCORES, T, 128, PK).transpose(0, 2, 1, 3)
    ).reshape(NCORES * 128, T, PK)


def _consts(plan):
    """Call-invariant (per-signature) inputs, device-resident."""
    T = plan["T"]
    bz_flat = np.zeros(T * 128, np.float32)
    for f in plan["fills"]:
        bz_flat[f["gslot"]] = f["binl"]
    bzf = np.ascontiguousarray(
        np.broadcast_to(bz_flat.reshape(T, 128).T[None], (NCORES, 128, T))
    ).reshape(NCORES * 128, T)

    cxs = np.empty((NCORES, 128, 32), np.float32)
    for c in range(NCORES):
        X0 = 32 * (c % 4)
        cxs[c] = ((np.arange(32, dtype=np.float32) + X0) + np.float32(0.5)) * H
    cxs = cxs.reshape(NCORES * 128, 32)

    iota = np.ascontiguousarray(
        np.broadcast_to(
            np.arange(132, dtype=np.float32), (NCORES * 128, 132)
        )
    ).astype(BF16NP)
    oc = np.ascontiguousarray(
        np.broadcast_to(
            np.arange(-2, 3).astype(np.float32) + 0.5, (NCORES * 128, 5)
        )
    )
    return dict(bzf=bzf, cxs=cxs, iota=iota, oyc=oc, ozc=oc.copy())


# ------------------------------------------------------------ bass program
CH = 8  # tiles per chunk


def _build_nc(plan):
    spline = _register_spline()
    T = plan["T"]
    nc = bacc.Bacc("TRN2", target_bir_lowering=False, debug=False, num_devices=NCORES)

    di = {}
    di["pk"] = nc.dram_tensor("pk", [128, T, PK], u8, kind="ExternalInput")
    di["bzf"] = nc.dram_tensor("bzf", [128, T], f32, kind="ExternalInput")
    di["cxs"] = nc.dram_tensor("cxs", [128, 32], f32, kind="ExternalInput")
    di["iota"] = nc.dram_tensor("iota", [128, 132], bf16, kind="ExternalInput")
    di["oyc"] = nc.dram_tensor("oyc", [128, 5], f32, kind="ExternalInput")
    di["ozc"] = nc.dram_tensor("ozc", [128, 5], f32, kind="ExternalInput")
    OUTQ = nc.dram_tensor("OUTQ", [32, 128, 516], u8, kind="ExternalOutput")

    Sq = mybir.ActivationFunctionType.Square
    Sqrt = mybir.ActivationFunctionType.Sqrt
    Copy = mybir.ActivationFunctionType.Copy
    AOp = mybir.AluOpType

    with tile.TileContext(nc) as tc:
        with (
            tc.tile_pool(name="ins", bufs=1) as ins,
            tc.tile_pool(name="work", bufs=2) as wk,
            tc.tile_pool(name="slabp", bufs=2) as slabp,
            tc.tile_pool(name="psum", bufs=8, space="PSUM") as psp,
        ):
            # resident inputs
            sb = {}
            sb["pk"] = ins.tile([128, T, PK], u8, tag="pk", name="pk")
            nc.sync.dma_start(sb["pk"][:], di["pk"][:])
            for nm, w, dt_ in (
                ("bzf", T, f32), ("cxs", 32, f32), ("iota", 132, bf16),
                ("oyc", 5, f32), ("ozc", 5, f32),
            ):
                sb[nm] = ins.tile([128, w], dt_, tag=nm, name=nm + "_sb")
                nc.sync.dma_start(sb[nm][:], di[nm][:])

            # unpack cdat bf16 bytes into a contiguous tile (SBUF->SBUF DMA)
            sb["cdat"] = ins.tile([128, T, C], bf16, tag="cdat", name="cdat_sb")
            nc.sync.dma_start(
                sb["cdat"][:].bitcast(u8), sb["pk"][:, :, 5:13]
            )
            sb["cdatf"] = ins.tile([128, T, C], f32, tag="cdatf", name="cdatf_sb")
            nc.scalar.copy(
                sb["cdatf"][:].rearrange("p t c -> p (t c)"),
                sb["cdat"][:].rearrange("p t c -> p (t c)"),
            )
            # decode u8 pack -> f32 positions
            pqs = ins.tile([128, T, 3], f32, tag="pqs", name="pqs_sb")
            nc.scalar.activation(
                pqs[:],
                sb["pk"][:, :, 0:3],
                Copy,
                scale=float(H / np.float32(255.0)),
            )
            sb["by2f"] = ins.tile([128, T], f32, tag="by2f", name="by2f_sb")
            nc.scalar.copy(sb["by2f"][:], sb["pk"][:, :, 4])
            sb["by2b"] = ins.tile([128, T], bf16, tag="by2b", name="by2b_sb")
            nc.scalar.copy(sb["by2b"][:], sb["pk"][:, :, 4])
            sb["byf"] = ins.tile([128, T], f32, tag="byf", name="byf_sb")
            nc.vector.tensor_scalar(
                out=sb["byf"][:], in0=sb["by2f"][:],
                scalar1=-2.0, scalar2=None, op0=AOp.add,
            )
            # px = bx*H + qx*(H/255); py = (by2-2)*H + qy*(H/255); pz analog
            bxH = ins.tile([128, T], f32, tag="bxH", name="bxH_sb")
            nc.scalar.activation(
                bxH[:], sb["pk"][:, :, 3], Copy, scale=float(H)
            )
            byH = ins.tile([128, T], f32, tag="byH", name="byH_sb")
            nc.vector.tensor_scalar(
                out=byH[:], in0=sb["by2f"][:],
                scalar1=float(H), scalar2=float(-2.0 * H),
                op0=AOp.mult, op1=AOp.add,
            )
            bzH = ins.tile([128, T], f32, tag="bzH", name="bzH_sb")
            nc.vector.tensor_scalar(
                out=bzH[:], in0=sb["bzf"][:],
                scalar1=float(H), scalar2=None, op0=AOp.mult,
            )
            for nm, bt, cidx in (("px", bxH, 0), ("py", byH, 1), ("pz", bzH, 2)):
                sb[nm] = ins.tile([128, T], f32, tag=nm, name=nm)
                nc.vector.tensor_tensor(
                    out=sb[nm][:], in0=bt[:], in1=pqs[:, :, cidx], op=AOp.add
                )

            for p in range(NPH):
                t_lo, t_hi = plan["phase_tiles"][p]
                ntile = t_hi - t_lo
                slab = slabp.tile([128, 8, 512], f32, tag="slab")
                nc.gpsimd.memset(slab[:], 0.0)

                # group units by chunk
                units_by_chunk = {}
                for z, t, r0, nr, zoff in plan["phase_units"][p]:
                    ci = (t - t_lo) // CH
                    units_by_chunk.setdefault(ci, []).append((z, t, r0, nr))

                nchunk = (ntile + CH - 1) // CH
                for ci in range(nchunk):
                    c_lo = t_lo + ci * CH
                    cw = min(CH, t_hi - c_lo)
                    sl = slice(c_lo, c_lo + cw)

                    # ---- A: axis deltas
                    dxa = wk.tile([128, CH, 8], f32, tag="dxa")
                    nc.vector.tensor_tensor(
                        out=dxa[:, :cw],
                        in0=sb["cxs"][:, None, 8 * p:8 * p + 8].to_broadcast(
                            [128, cw, 8]
                        ),
                        in1=sb["px"][:, sl, None].to_broadcast([128, cw, 8]),
                        op=AOp.subtract,
                    )
                    dxa2 = wk.tile([128, CH, 8], f32, tag="dxa2")
                    nc.scalar.activation(dxa2[:, :cw], dxa[:, :cw], Sq)

                    ty = wk.tile([128, CH, 5], f32, tag="ty")
                    nc.vector.tensor_tensor(
                        out=ty[:, :cw],
                        in0=sb["byf"][:, sl, None].to_broadcast([128, cw, 5]),
                        in1=sb["oyc"][:, None, :].to_broadcast([128, cw, 5]),
                        op=AOp.add,
                    )
                    dy = wk.tile([128, CH, 5], f32, tag="dy")
                    nc.vector.scalar_tensor_tensor(
                        out=dy[:, :cw],
                        in0=ty[:, :cw],
                        scalar=float(H),
                        in1=sb["py"][:, sl, None].to_broadcast([128, cw, 5]),
                        op0=AOp.mult,
                        op1=AOp.subtract,
                    )
                    dy2 = wk.tile([128, CH, 5], f32, tag="dy2")
                    nc.scalar.activation(dy2[:, :cw], dy[:, :cw], Sq)

                    tz = wk.tile([128, CH, 5], f32, tag="tz")
                    nc.vector.tensor_tensor(
                        out=tz[:, :cw],
                        in0=sb["bzf"][:, sl, None].to_broadcast([128, cw, 5]),
                        in1=sb["ozc"][:, None, :].to_broadcast([128, cw, 5]),
                        op=AOp.add,
                    )
                    dz = wk.tile([128, CH, 5], f32, tag="dz")
                    nc.vector.scalar_tensor_tensor(
                        out=dz[:, :cw],
                        in0=tz[:, :cw],
                        scalar=float(H),
                        in1=sb["pz"][:, sl, None].to_broadcast([128, cw, 5]),
                        op0=AOp.mult,
                        op1=AOp.subtract,
                    )
                    dz2 = wk.tile([128, CH, 5], f32, tag="dz2")
                    nc.scalar.activation(dz2[:, :cw], dz[:, :cw], Sq)

                    # ---- B: d2 in (oy, xs, oz) order
                    tyx = wk.tile([128, CH, 5, 8], f32, tag="tyx")
                    nc.vector.tensor_tensor(
                        out=tyx[:, :cw],
                        in0=dy2[:, :cw, :, None].to_broadcast([128, cw, 5, 8]),
                        in1=dxa2[:, :cw, None, :].to_broadcast([128, cw, 5, 8]),
                        op=AOp.add,
                    )
                    d2 = wk.tile([128, CH, 40, 5], f32, tag="d2")
                    nc.vector.tensor_tensor(
                        out=d2[:, :cw],
                        in0=tyx[:, :cw].rearrange("p t a b -> p t (a b)")[
                            :, :, :, None
                        ].to_broadcast([128, cw, 40, 5]),
                        in1=dz2[:, :cw, None, :].to_broadcast([128, cw, 40, 5]),
                        op=AOp.add,
                    )

                    # ---- C: q on ACT
                    d2f = d2[:, :cw].rearrange("p t a b -> p (t a b)")
                    qt = wk.tile([128, CH, 200], f32, tag="qt")
                    qf = qt[:, :cw].rearrange("p t s -> p (t s)")
                    nc.scalar.activation(qf, d2f, Sqrt, scale=25.0)
                    # ---- D: spline -> W bf16 (q^2 == 25*d2 exactly)
                    Wt = wk.tile([128, CH, 200], bf16, tag="Wt")
                    nc.vector._custom_dve(
                        spline,
                        out=Wt[:, :cw].rearrange("p t s -> p (t s)"),
                        in0=qf,
                        in1=d2f,
                        s0=75.0,
                        s1=0.0,
                        imm2=0.5,
                    )

                    # ---- E: vals[k,t,c,spl] = W * cdat_c  (c-major; TS hits
                    # 4x bf16 mode on DVE; remainder on ACT Copy-scale)
                    vals = wk.tile([128, CH, C, 200], bf16, tag="vals")
                    esplit = min(cw, 3)
                    for tl in range(cw):
                        for cc in range(C):
                            if tl < esplit:
                                nc.vector.tensor_scalar(
                                    out=vals[:, tl, cc],
                                    in0=Wt[:, tl],
                                    scalar1=sb["cdatf"][:, c_lo + tl, cc, None],
                                    scalar2=None,
                                    op0=AOp.mult,
                                )
                            else:
                                nc.scalar.activation(
                                    out=vals[:, tl, cc],
                                    in_=Wt[:, tl],
                                    func=Copy,
                                    scale=sb["cdatf"][:, c_lo + tl, cc, None],
                                )

                    # ---- onehot (per-tile TS is_equal; 4x bf16 mode)
                    oh = wk.tile([128, CH, 132], bf16, tag="oh")
                    for tl in range(cw):
                        nc.vector.tensor_scalar(
                            out=oh[:, tl],
                            in0=sb["iota"][:],
                            scalar1=sb["by2f"][:, c_lo + tl, None],
                            scalar2=None,
                            op0=AOp.is_equal,
                        )

                    # ---- F: matmuls + evac per unit
                    for z, t, r0, nr in units_by_chunk.get(ci, []):
                        tl = t - c_lo
                        ps = psp.tile([128, 160], f32, tag="ps", name="ps")
                        for oyi in range(5):
                            c0 = 2 - (oyi - 2)
                            nc.tensor.matmul(
                                out=ps[:],
                                lhsT=oh[r0:r0 + nr, tl, c0:c0 + 128],
                                rhs=vals[
                                    r0:r0 + nr, tl, :,
                                    40 * oyi:40 * (oyi + 1)
                                ],
                                start=(oyi == 0),
                                stop=(oyi == 4),
                                tile_position=(r0, 0) if r0 >= 96 else None,
                            )
                        # evac with z-clip (cell granularity)
                        oz_lo = max(0, (8 - 4 * z) // 4)
                        oz_hi = min(5, (512 - (4 * z - 8)) // 4)
                        nz = oz_hi - oz_lo
                        zlo = 4 * z - 8 + 4 * oz_lo
                        sview = slab[:, :, zlo:zlo + 4 * nz].rearrange(
                            "p x (w c) -> p x w c", c=4
                        )
                        psr = ps[:].rearrange("p (c x w) -> p c x w", c=4, x=8)
                        pview = psr[:, :, :, oz_lo:oz_hi].rearrange(
                            "p c x w -> p x w c"
                        )
                        nc.vector.tensor_tensor(
                            out=sview, in0=sview, in1=pview, op=AOp.add
                        )

                # ---- phase out: int8 quantize with per-(y,x) block scale.
                # q = RNE(v * 127/absmax + 127)  (convert is RNE, probed);
                # host decodes v = (q - 127) * absmax/127. Integer offset so
                # the reconstruction grid contains 0 exactly (most grid cells
                # are near zero; a half-quantum offset would 5x the error).
                absm = slabp.tile([128, 8], f32, tag="absm")
                nc.vector.tensor_reduce(
                    out=absm[:], in_=slab[:], axis=mybir.AxisListType.X,
                    op=AOp.max, apply_absolute_value=True,
                )
                absc = slabp.tile([128, 8], f32, tag="absc")
                nc.vector.tensor_scalar(
                    out=absc[:], in0=absm[:],
                    scalar1=1e-30, scalar2=None, op0=AOp.max,
                )
                scl = slabp.tile([128, 8], f32, tag="scl")
                nc.vector.reciprocal(scl[:], absc[:])
                nc.vector.tensor_scalar(
                    out=scl[:], in0=scl[:],
                    scalar1=127.0, scalar2=None, op0=AOp.mult,
                )
                yt = slabp.tile([128, 8, 512], f32, tag="yt")
                nc.vector.tensor_tensor(
                    out=yt[:], in0=slab[:],
                    in1=scl[:, :, None].to_broadcast([128, 8, 512]),
                    op=AOp.mult,
                )
                q8 = slabp.tile([128, 8, 512], u8, tag="q8")
                nc.scalar.activation(
                    q8[:].rearrange("p x z -> p (x z)"),
                    yt[:].rearrange("p x z -> p (x z)"),
                    Copy, bias=127.0,
                )
                for xs in range(8):
                    nc.sync.dma_start(
                        out=OUTQ[8 * p + xs, :, 0:512], in_=q8[:, xs, :]
                    )
                # scales ride in the last 4 byte-columns of OUTQ
                nc.sync.dma_start(
                    out=OUTQ[8 * p:8 * p + 8, :, 512:516].rearrange(
                        "x p c -> p x c"
                    ),
                    in_=absm[:].bitcast(u8).rearrange("p (x c) -> p x c", c=4),
                )
    nc.compile()
    return nc


# ------------------------------------------------------------------ runner
def _make_runner(nc, consts):
    """Cached PJRT runner: replicates bass2jax.run_bass_via_pjrt but hoists
    the jitted shard_map (trace/compile once), recycles the previous call's
    output buffers as the donated outputs (outputs are fully overwritten),
    and keeps call-invariant inputs device-resident. dispatch() is async:
    it returns a dict of global jax Arrays (fetch with np.asarray)."""
    import jax
    import jax.numpy as jnp
    from jax.experimental.shard_map import shard_map
    from jax.sharding import Mesh, PartitionSpec, NamedSharding
    from concourse import bass2jax
    from concourse.bass2jax import _bass_exec_p, partition_id_tensor

    bass2jax.install_neuronx_cc_hook()
    assert nc.dbg_addr is None, "build with debug=False"
    pname = nc.partition_id_tensor.name if nc.partition_id_tensor is not None else None

    in_names, out_names, out_avals = [], [], []
    for alloc in nc.m.functions[0].allocations:
        if not isinstance(alloc, mybir.MemoryLocationSet):
            continue
        name = alloc.memorylocations[0].name
        if alloc.kind == "ExternalInput":
            if name != pname:
                in_names.append(name)
        elif alloc.kind == "ExternalOutput":
            out_names.append(name)
            out_avals.append(
                jax.core.ShapedArray(
                    tuple(alloc.tensor_shape), mybir.dt.np(alloc.dtype)
                )
            )
    n_params = len(in_names)
    n_outs = len(out_names)
    bind_names = tuple(in_names + out_names + ([pname] if pname else []))
    donate = tuple(range(n_params, n_params + n_outs))

    def _body(*args):
        operands = list(args)
        if pname is not None:
            operands.append(partition_id_tensor())
        outs = _bass_exec_p.bind(
            *operands,
            out_avals=tuple(out_avals),
            in_names=bind_names,
            out_names=tuple(out_names),
            lowering_input_output_aliases=(),
            sim_require_finite=True,
            sim_require_nnan=True,
            nc=nc,
        )
        return tuple(outs)

    devices = jax.devices()[:NCORES]
    assert len(devices) == NCORES
    mesh = Mesh(np.asarray(devices), ("core",))
    P_ = PartitionSpec("core")
    sharded = jax.jit(
        shard_map(
            _body,
            mesh=mesh,
            in_specs=(P_,) * (n_params + n_outs),
            out_specs=(P_,) * n_outs,
            check_rep=False,
        ),
        donate_argnums=donate,
        keep_unused=True,
    )
    gsh = NamedSharding(mesh, P_)
    zshapes = [(NCORES * a.shape[0], *a.shape[1:]) for a in out_avals]
    zdtypes = [a.dtype for a in out_avals]
    zfn = jax.jit(
        lambda: tuple(jnp.zeros(s, d) for s, d in zip(zshapes, zdtypes)),
        out_shardings=(gsh,) * n_outs,
    )

    const_dev = {nm: jax.device_put(arr, gsh) for nm, arr in consts.items()}
    state = {"prev": None}

    def dispatch(var_inputs):
        dev_in = {
            nm: jax.device_put(a, gsh) for nm, a in var_inputs.items()
        }
        args = [
            const_dev[nm] if nm in const_dev else dev_in[nm]
            for nm in in_names
        ]
        seeds = state["prev"] if state["prev"] is not None else zfn()
        outs = sharded(*args, *seeds)
        state["prev"] = outs
        return {nm: o for nm, o in zip(out_names, outs)}

    return dispatch


_DEC_LUT = (np.arange(256, dtype=np.float32) - np.float32(127.0))


def _decode(arr, out6, ex):
    """Threaded decode of [256,128,516] u8 (q8 + embedded f32 scales) into
    out6 = out viewed as [2,4,32,128,512]. Returns futures."""
    scales = arr[:, :, 512:516].copy().view(np.float32)[..., 0]  # [256,128]
    fac = (scales.astype(np.float64) / 127.0).astype(np.float32)[..., None]

    def conv(ci):
        b, qq = ci // 4, ci % 4
        r = slice(ci * 32, ci * 32 + 32)
        blk = arr[r, :, :512].astype(np.float32)
        blk -= 127.0
        blk *= fac[r]
        out6[b, qq] = blk

    return [ex.submit(conv, ci) for ci in range(NCORES)]


# ------------------------------------------------------------------ driver
_CACHE = {}


def kernel(locs, data, density):
    import time as _time
    from concurrent.futures import ThreadPoolExecutor

    t0 = _time.time()
    locs = np.asarray(locs)
    data = np.asarray(data)
    density = np.asarray(density)
    plan = _build_plan(locs, data, density)
    t1 = _time.time()
    entry = _CACHE.get(plan["sig"])
    if entry is None:
        entry = {"dispatch": _make_runner(_build_nc(plan), _consts(plan))}
        _CACHE[plan["sig"]] = entry
    t2 = _time.time()

    pk = _fill_all(plan)
    outs = entry["dispatch"]({"pk": pk})
    t3 = _time.time()

    out = np.empty((B, GS, GS, GS, C), np.float32)
    out6 = out.reshape(2, 4, 32, 128, 512)
    arr = np.asarray(outs["OUTQ"])               # ONE gather, blocks on exec
    t4 = _time.time()
    with ThreadPoolExecutor(NCORES) as ex:
        for f in _decode(arr, out6, ex):
            f.result()
    t5 = _time.time()
    print(
        f"[kernel] plan={t1-t0:.3f}s build={t2-t1:.3f}s fill+disp={t3-t2:.3f}s "
        f"gather={t4-t3:.3f}s dec={t5-t4:.3f}s T={plan['T']}"
    )
    return out
